# revision 14
# baseline (speedup 1.0000x reference)
"""BuildingGCN Trainium2 kernel v2: 3-layer GCN + global mean pool + MLP head,
distributed over 8 NeuronCores.

Channel-major ("transposed") dataflow: aggregation output S^T [c, dst] is
computed as sum_b gt_b[:, :c].T @ band_b where gt blocks are dma_gathered
rows of a bf16 y-table (256B rows, AllGathered per layer) and band_b is the
0/1 segment matrix generated on DVE via tensor_scalar is_equal (4x mode).
Aggregation space per layer: L1 in 8ch (agg-first), L2 in 64ch (agg-first),
L3 in 128ch (agg-first). Gathers run on all 4 SWDGE queues in parallel
(~2.7ns/idx vs 8.6 on one queue). Per-node scales (dinv) are applied with a
host-replicated [128, NS] tile; the tiny MLP head is replicated.
"""
import sys
import types
from dataclasses import dataclass, field

import numpy as np
import ml_dtypes

import concourse.bass as bass
import concourse.tile as tile
from concourse import bacc, mybir
from concourse._compat import cdiv
from concourse.bass_utils import run_bass_kernel_spmd

P = 128
F32 = mybir.dt.float32
BF16 = mybir.dt.bfloat16
I16 = mybir.dt.int16
BF = ml_dtypes.bfloat16

N_NODES = 100000
N_CORES = 8
N_GRAPHS = 256
CHUNK = 25000
GW = 256          # dst group width (2 windows)
IN_CH = 8
C1, C2, C3 = 64, 128, 64
CH1 = 32
GSLOTS = 64


@dataclass
class Cfg:
    n_nodes: int = N_NODES
    n_cores: int = N_CORES

    @property
    def ns(self):
        return self.n_nodes // self.n_cores          # 12500

    @property
    def nw(self):
        return cdiv(self.ns, P)                       # 98

    @property
    def ng(self):
        return cdiv(self.ns, GW)                      # 49

    @property
    def n_chunks(self):
        return cdiv(self.n_nodes, CHUNK)              # 4

    @property
    def npad(self):
        return self.nw * P                            # 12544


@dataclass
class Plan:
    cfg: Cfg
    nb: np.ndarray                 # [NG, 4] blocks per cell (max over cores)
    g0: list = field(default_factory=list)
    gwid: list = field(default_factory=list)
    icol0: np.ndarray = None       # idx col offset per cell
    b0: np.ndarray = None          # seg block offset per cell
    idx_cols: int = 0
    nb_total: int = 0

    def finalize(self):
        ng, nk = self.nb.shape
        self.icol0 = np.zeros((ng, nk), dtype=np.int64)
        self.b0 = np.zeros((ng, nk), dtype=np.int64)
        icol = 0
        b = 0
        for g in range(ng):
            for k in range(nk):
                self.icol0[g, k] = icol
                self.b0[g, k] = b
                icol += int(self.nb[g, k]) * P // 16
                b += int(self.nb[g, k])
        self.idx_cols = icol
        self.nb_total = b


def preprocess(cfg: Cfg, edge_index, batch):
    src = np.asarray(edge_index[0], dtype=np.int64)
    dst = np.asarray(edge_index[1], dtype=np.int64)
    batch = np.asarray(batch, dtype=np.int64)
    NS, NG, NK = cfg.ns, cfg.ng, cfg.n_chunks

    deg = (np.bincount(dst, minlength=cfg.n_nodes) + 1.0).astype(np.float32)
    dinv = (1.0 / np.sqrt(deg)).astype(np.float32)
    cnt = np.bincount(batch, minlength=N_GRAPHS).astype(np.float32)
    invcnt = (1.0 / np.maximum(cnt, 1.0)).astype(np.float32)

    per_core = []
    counts = np.zeros((cfg.n_cores, NG, NK), dtype=np.int64)
    for c in range(cfg.n_cores):
        m = (dst >= c * NS) & (dst < (c + 1) * NS)
        s = src[m]
        d = dst[m] - c * NS
        g = d // GW
        k = s // CHUNK
        key = g * NK + k
        order = np.argsort(key, kind="stable")
        s, d, key = s[order], d[order], key[order]
        cnts = np.bincount(key, minlength=NG * NK).reshape(NG, NK)
        counts[c] = cnts
        starts = np.zeros(NG * NK + 1, dtype=np.int64)
        np.cumsum(cnts.reshape(-1), out=starts[1:])
        per_core.append((s, d, starts))

    nb = np.ceil(counts.max(axis=0) / P).astype(np.int64)   # [NG, NK]
    plan = Plan(Cfg(), nb)
    plan.g0 = [int(batch[c * NS]) for c in range(cfg.n_cores)]
    plan.gwid = [int(batch[(c + 1) * NS - 1]) - plan.g0[c] + 1
                 for c in range(cfg.n_cores)]
    assert max(plan.gwid) <= GSLOTS
    plan.finalize()

    shared = {
        "iota256": np.tile(np.arange(GW, dtype=np.float32),
                           (P, 1)).astype(BF),
        "iota64": np.tile(np.arange(GSLOTS, dtype=np.float32),
                          (P, 1)),
        "ident": np.eye(P, dtype=np.float32),
        "invcnt": np.tile(invcnt, (C3, 1)).astype(np.float32).reshape(
            C3, N_GRAPHS),
    }

    cores = []
    for c in range(cfg.n_cores):
        s, d, starts = per_core[c]
        idx_stream = np.zeros(plan.idx_cols * 16, dtype=np.int16)
        seg_stream = np.full(plan.nb_total * P, -1.0, dtype=np.float32)
        for g in range(NG):
            for k in range(NK):
                e0, e1 = starts[g * NK + k], starts[g * NK + k + 1]
                n = e1 - e0
                ipos = int(plan.icol0[g, k]) * 16
                bpos = int(plan.b0[g, k]) * P
                idx_stream[ipos:ipos + n] = (s[e0:e1] % CHUNK).astype(np.int16)
                seg_stream[bpos:bpos + n] = (d[e0:e1] % GW).astype(np.float32)
        idx_all = np.tile(idx_stream.reshape(-1, 16).T, (8, 1))
        seg_all = seg_stream.reshape(-1, P).T.copy()

        nodes = np.arange(c * NS, (c + 1) * NS)
        dv = np.ones(cfg.npad, dtype=np.float32)
        dv[:NS] = dinv[nodes]
        dinv_rep = np.tile(dv, (P, 1)).astype(BF)
        gs = np.full(cfg.npad, -1.0, dtype=np.float32)
        gs[:NS] = (batch[nodes] - plan.g0[c]).astype(np.float32)
        gslot_t = gs.reshape(cfg.nw, P).T.copy()
        cores.append({
            "idx_all": idx_all, "seg_all": seg_all,
            "dinv_rep": dinv_rep, "gslot_t": gslot_t,
        })
    return plan, shared, cores


def build_program(plan: Plan, n_cores: int):
    cfg = plan.cfg
    NG, NK, NS, NW = cfg.ng, cfg.n_chunks, cfg.ns, cfg.nw
    nc = bacc.Bacc("TRN2", target_bir_lowering=False, debug=False,
                   num_devices=n_cores, num_swdge_queues=4)

    def din(name, shape, dt=F32):
        return nc.dram_tensor(name, shape, dt, kind="ExternalInput").ap()

    xT = din("xT", [IN_CH, cfg.npad])
    w1 = din("w1", [IN_CH, C1])
    w2 = din("w2", [C1, C2])
    w3 = din("w3", [C2, C3])
    wl1 = din("wl1", [C3, CH1])
    wl2 = din("wl2", [CH1, 1])
    b1c = din("b1c", [C1, 1])
    b2c = din("b2c", [C2, 1])
    b3c = din("b3c", [C3, 1])
    bl1c = din("bl1c", [CH1, 1])
    bl2c = din("bl2c", [1, 1])
    dinv_d = din("dinv_rep", [P, cfg.npad], BF16)
    gslot_d = din("gslot_t", [P, NW])
    iota256_d = din("iota256", [P, GW], BF16)
    iota64_d = din("iota64", [P, GSLOTS])
    ident_d = din("ident", [P, P])
    invcnt_d = din("invcnt", [C3, N_GRAPHS])
    idx_d = din("idx_all", [P, plan.idx_cols], I16)
    seg_d = din("seg_all", [P, plan.nb_total])
    out_d = nc.dram_tensor("out", [1, N_GRAPHS], F32,
                           kind="ExternalOutput").ap()
    DBG = bool(int(__import__("os").environ.get("GCN_DBG", "0")))
    if DBG:
        dbg_tab1 = nc.dram_tensor("dbg_tab1", [NS, P], BF16,
                                  kind="ExternalOutput").ap()
        nb00 = int(plan.nb[0, 0])
        dbg_gt = nc.dram_tensor("dbg_gt", [P, nb00, P], BF16,
                                kind="ExternalOutput").ap()
        dbg_band = nc.dram_tensor("dbg_band", [P, nb00, GW], BF16,
                                  kind="ExternalOutput").ap()
        dbg_ps = nc.dram_tensor("dbg_ps", [IN_CH, GW], F32,
                                kind="ExternalOutput").ap()
        dbg_yres = nc.dram_tensor("dbg_yres", [P, 12544], BF16,
                                  kind="ExternalOutput").ap()

    with tile.TileContext(nc) as tc:
        with tc.tile_pool(name="const", bufs=1) as cp, \
             tc.tile_pool(name="gath", bufs=5) as gp, \
             tc.tile_pool(name="band", bufs=3) as bp, \
             tc.tile_pool(name="win", bufs=2) as wp, \
             tc.tile_pool(name="psS", bufs=2, space="PSUM") as pps, \
             tc.tile_pool(name="psW", bufs=2, space="PSUM") as ppw, \
             tc.tile_pool(name="psT", bufs=1, space="PSUM") as ppt, \
             tc.tile_pool(name="psP", bufs=1, space="PSUM") as ppp, \
             tc.tile_pool(name="dram", bufs=1, space="DRAM") as dp:

            def load_const(name, ap, shape, dt=F32):
                t = cp.tile(shape, dt, tag=name)
                nc.sync.dma_start(t[:], ap[:])
                return t

            # f32 weights -> bf16 SBUF tiles
            def wbf(name, ap, shape):
                t32 = wp.tile(shape, F32, tag="wtmp")
                nc.sync.dma_start(t32[:], ap[:])
                t = cp.tile(shape, BF16, tag=name)
                nc.vector.tensor_copy(t[:], t32[:])
                return t

            w1_s = load_const("w1", w1, [IN_CH, C1])
            w2_s = load_const("w2", w2, [C1, C2])
            w3_s = load_const("w3", w3, [C2, C3])
            wl1_s = load_const("wl1", wl1, [C3, CH1])
            wl2_s = load_const("wl2", wl2, [CH1, 1])
            b1_s = load_const("b1", b1c, [C1, 1])
            b2_s = load_const("b2", b2c, [C2, 1])
            b3_s = load_const("b3", b3c, [C3, 1])
            bl1_s = load_const("bl1", bl1c, [CH1, 1])
            bl2_s = load_const("bl2", bl2c, [1, 1])
            dinv_s = load_const("dinv", dinv_d, [P, cfg.npad], BF16)
            gslot_s = load_const("gslot", gslot_d, [P, NW])
            iota256_s = load_const("iota256", iota256_d, [P, GW], BF16)
            iota64_s = load_const("iota64", iota64_d, [P, GSLOTS])
            ident_s = load_const("ident", ident_d, [P, P])
            invcnt_s = load_const("invcnt", invcnt_d, [C3, N_GRAPHS])
            xT_s = load_const("xT", xT, [IN_CH, cfg.npad])

            yres = cp.tile([P, cfg.npad], F32)
            nc.vector.memset(yres[:], 0.0)

            tab_sh = dp.tile([NS, P], BF16)
            tab_full = dp.tile([cfg.n_nodes, P], BF16)
            pool_sh = dp.tile([C3, GSLOTS], F32)
            pool_ag = dp.tile([C3 * n_cores, GSLOTS], F32)
            rg = [list(range(n_cores))]

            def ts(out, in0, s1, op0, s2=None, op1=mybir.AluOpType.bypass):
                nc.vector.tensor_scalar(out, in0, s1, s2, op0, op1)

            def store_table(cin, g):
                """Transpose yres[:cin, group cols] into tab_sh rows."""
                for wl in range(2):
                    w = g * 2 + wl
                    if w >= NW:
                        break
                    rows = NS - w * P if w == NW - 1 else P
                    tp = ppt.tile([P, P], F32, tag="tp")
                    nc.tensor.transpose(
                        tp[:, :cin], yres[:cin, w * P:(w + 1) * P],
                        ident_s[:cin, :cin])
                    tsb = wp.tile([P, P], BF16, tag="tsb")
                    nc.scalar.copy(tsb[:, :cin], tp[:, :cin])
                    nc.sync.dma_start(tab_sh[w * P:w * P + rows, :cin],
                                      tsb[:rows, :cin])

            def allgather():
                nc.gpsimd.collective_compute(
                    "AllGather", mybir.AluOpType.bypass, replica_groups=rg,
                    ins=[tab_sh.opt()], outs=[tab_full.opt()])

            # ---- prep: y1 = dinv * x (8ch), table1, AG ----
            nc.vector.tensor_tensor(yres[:IN_CH, :], xT_s[:],
                                    dinv_s[:IN_CH, :], mybir.AluOpType.mult)
            for g in range(NG):
                store_table(IN_CH, g)
            if DBG:
                nc.sync.dma_start(dbg_tab1[:], tab_sh[:])
            allgather()

            pool_ps = ppp.tile([C3, GSLOTS], F32)

            def agg_layer(lnum, cin, cout, w_s, b_s):
                last_layer = lnum == 3
                for g in range(NG):
                    ps = pps.tile([cin, GW], F32, tag="psS")
                    nblist = [int(plan.nb[g, k]) for k in range(NK)]
                    tot = sum(nblist)
                    bi = 0
                    for k in range(NK):
                        nbk = nblist[k]
                        if nbk == 0:
                            continue
                        ni = nbk * P
                        cols = ni // 16
                        c0 = int(plan.icol0[g, k])
                        it = wp.tile([P, cols], I16, tag=f"idx{k}")
                        nc.sync.dma_start(it[:], idx_d[:, c0:c0 + cols])
                        gt = gp.tile([P, nbk, P], BF16, tag="gt")
                        rlo = k * CHUNK
                        rhi = min(rlo + CHUNK, cfg.n_nodes)
                        nc.gpsimd.dma_gather(
                            gt[:], tab_full[rlo:rhi, :], it[:],
                            ni, ni, P, single_packet=False, queue_num=k)
                        sb0 = int(plan.b0[g, k])
                        seg_t = wp.tile([P, nbk], F32, tag=f"seg{k}")
                        nc.sync.dma_start(seg_t[:], seg_d[:, sb0:sb0 + nbk])
                        band = bp.tile([P, nbk, GW], BF16, tag="band")
                        for b in range(nbk):
                            ts(band[:, b, :], iota256_s[:], seg_t[:, b:b + 1],
                               mybir.AluOpType.is_equal)
                        if DBG and lnum == 1 and g == 0 and k == 0:
                            nc.sync.dma_start(dbg_gt[:], gt[:])
                            nc.sync.dma_start(dbg_band[:], band[:])
                        for b in range(nbk):
                            nc.tensor.matmul(
                                ps[:], lhsT=gt[:, b, :cin], rhs=band[:, b, :],
                                start=(bi == 0), stop=(bi == tot - 1),
                                skip_group_check=True)
                            bi += 1
                    # post: u = dinv * (S^T + y^T); h = relu(W^T u + b)
                    sl = slice(g * GW, (g + 1) * GW)
                    if DBG and lnum == 1 and g == 0:
                        pscp = wp.tile([IN_CH, GW], F32, tag="pscp")
                        nc.vector.tensor_copy(pscp[:], ps[:])
                        nc.sync.dma_start(dbg_ps[:], pscp[:])
                    t2 = wp.tile([cin, GW], F32, tag="t2")
                    nc.vector.tensor_tensor(t2[:], ps[:], yres[:cin, sl],
                                            mybir.AluOpType.add)
                    u = wp.tile([cin, GW], F32, tag="u")
                    nc.vector.tensor_tensor(u[:], t2[:], dinv_s[:cin, sl],
                                            mybir.AluOpType.mult)
                    ps2 = ppw.tile([cout, GW], F32, tag="psW")
                    nc.tensor.matmul(ps2[:], lhsT=w_s[:], rhs=u[:],
                                     start=True, stop=True)
                    h = wp.tile([cout, GW], F32, tag="h")
                    nc.scalar.activation(
                        h[:], ps2[:], mybir.ActivationFunctionType.Relu,
                        bias=b_s[:, 0:1])
                    if not last_layer:
                        nc.vector.tensor_tensor(
                            yres[:cout, sl], h[:], dinv_s[:cout, sl],
                            mybir.AluOpType.mult)
                        store_table(cout, g)
                    else:
                        for wl in range(2):
                            w = g * 2 + wl
                            if w >= NW:
                                break
                            tp = ppt.tile([P, P], F32, tag="tp")
                            nc.tensor.transpose(
                                tp[:, :cout], h[:, wl * P:(wl + 1) * P],
                                ident_s[:cout, :cout])
                            h3w = wp.tile([P, C3], F32, tag="h3w")
                            nc.scalar.copy(h3w[:], tp[:, :cout])
                            gb = wp.tile([P, GSLOTS], F32, tag="gb")
                            ts(gb[:], iota64_s[:], gslot_s[:, w:w + 1],
                               mybir.AluOpType.is_equal)
                            nc.tensor.matmul(
                                pool_ps[:], lhsT=h3w[:], rhs=gb[:],
                                start=(w == 0), stop=(w == NW - 1),
                                skip_group_check=True)

            agg_layer(1, IN_CH, C1, w1_s, b1_s)
            if DBG:
                nc.sync.dma_start(dbg_yres[:], yres[:])
            allgather()
            agg_layer(2, C1, C2, w2_s, b2_s)
            allgather()
            agg_layer(3, C2, C3, w3_s, b3_s)

            # ---- pooling finale + MLP (replicated) ----
            pool_sb = cp.tile([C3, GSLOTS], F32)
            nc.vector.tensor_copy(pool_sb[:], pool_ps[:])
            nc.sync.dma_start(pool_sh[:], pool_sb[:])
            nc.gpsimd.collective_compute(
                "AllGather", mybir.AluOpType.bypass, replica_groups=rg,
                ins=[pool_sh.opt()], outs=[pool_ag.opt()])
            M = cp.tile([C3, N_GRAPHS], F32)
            nc.vector.memset(M[:], 0.0)
            for c in range(n_cores):
                agc = wp.tile([C3, GSLOTS], F32, tag="agc")
                nc.sync.dma_start(agc[:], pool_ag[c * C3:(c + 1) * C3, :])
                g0 = plan.g0[c]
                wdt = min(plan.gwid[c], N_GRAPHS - g0)
                nc.vector.tensor_tensor(M[:, g0:g0 + wdt], M[:, g0:g0 + wdt],
                                        agc[:, :wdt], mybir.AluOpType.add)
            Mb = cp.tile([C3, N_GRAPHS], F32)
            nc.vector.tensor_tensor(Mb[:], M[:], invcnt_s[:],
                                    mybir.AluOpType.mult)
            ps1 = ppw.tile([CH1, N_GRAPHS], F32, tag="psW")
            nc.tensor.matmul(ps1[:], lhsT=wl1_s[:], rhs=Mb[:],
                             start=True, stop=True)
            g1 = cp.tile([CH1, N_GRAPHS], F32)
            nc.scalar.activation(g1[:], ps1[:],
                                 mybir.ActivationFunctionType.Relu,
                                 bias=bl1_s[:, 0:1])
            ps2 = ppw.tile([1, N_GRAPHS], F32, tag="psW")
            nc.tensor.matmul(ps2[:], lhsT=wl2_s[:], rhs=g1[:],
                             start=True, stop=True)
            osb = cp.tile([1, N_GRAPHS], F32)
            ts(osb[:], ps2[:], bl2_s[:, 0:1], mybir.AluOpType.add)
            nc.sync.dma_start(out_d[:], osb[:])

    nc.compile()
    return nc


def make_in_maps(cfg, plan, shared, cores, x, W1, b1, W2, b2, W3, b3,
                 Wl1, bl1, Wl2, bl2):
    NS = cfg.ns
    x = np.asarray(x, dtype=np.float32)
    com = {
        "w1": np.asarray(W1, np.float32), "w2": np.asarray(W2, np.float32),
        "w3": np.asarray(W3, np.float32),
        "wl1": np.asarray(Wl1, np.float32), "wl2": np.asarray(Wl2, np.float32),
        "b1c": np.asarray(b1, np.float32).reshape(-1, 1),
        "b2c": np.asarray(b2, np.float32).reshape(-1, 1),
        "b3c": np.asarray(b3, np.float32).reshape(-1, 1),
        "bl1c": np.asarray(bl1, np.float32).reshape(-1, 1),
        "bl2c": np.asarray(bl2, np.float32).reshape(1, 1),
        "iota256": shared["iota256"], "iota64": shared["iota64"],
        "ident": shared["ident"], "invcnt": shared["invcnt"],
    }
    in_maps = []
    for c in range(cfg.n_cores):
        m = dict(com)
        xs = np.zeros((IN_CH, cfg.npad), dtype=np.float32)
        xs[:, :NS] = x[c * NS:(c + 1) * NS].T
        m["xT"] = xs
        m.update(cores[c])
        in_maps.append(m)
    return in_maps


_CACHE = {}


def _install_profile_hook():
    try:
        import antenv.axon_hooks  # noqa: F401
        return
    except ImportError:
        pass
    try:
        mod = types.ModuleType("antenv.axon_hooks")
        _h = [None]
        mod.set_axon_ntff_profile_hook = lambda h: _h.__setitem__(0, h)
        mod.get_axon_ntff_profile_hook = lambda: _h[0]
        sys.modules["antenv.axon_hooks"] = mod
        from trn_agent_boot.trn_boot import _ntff_profile_via_ctypes
        mod.set_axon_ntff_profile_hook(
            _ntff_profile_via_ctypes("/opt/axon/libaxon_pjrt.so"))
    except Exception:
        pass


def run(cfg, x, edge_index, batch, W1, b1, W2, b2, W3, b3, Wl1, bl1, Wl2, bl2,
        trace=False):
    plan, shared, cores = preprocess(cfg, edge_index, batch)
    key = ("prog", plan.nb_total, plan.idx_cols,
           tuple(plan.g0), tuple(plan.gwid))
    if key not in _CACHE:
        _CACHE[key] = build_program(plan, cfg.n_cores)
    nc = _CACHE[key]
    in_maps = make_in_maps(cfg, plan, shared, cores, x, W1, b1, W2, b2,
                           W3, b3, Wl1, bl1, Wl2, bl2)
    if trace:
        _install_profile_hook()
    res = run_bass_kernel_spmd(nc, in_maps, list(range(cfg.n_cores)),
                               trace=trace)
    out = np.asarray(res.results[0]["out"]).reshape(-1)[:N_GRAPHS]
    return out.astype(np.float32), res


def kernel(x, edge_index, batch, W1, b1, W2, b2, W3, b3, Wl1, bl1, Wl2, bl2):
    cfg = Cfg()
    out, _ = run(cfg, x, edge_index, batch, W1, b1, W2, b2, W3, b3,
                 Wl1, bl1, Wl2, bl2)
    return out


# revision 15
# speedup vs baseline: 1.9734x; 1.9734x over previous
"""BuildingGCN Trainium2 kernel v2: 3-layer GCN + global mean pool + MLP head,
distributed over 8 NeuronCores.

Channel-major ("transposed") dataflow: aggregation output S^T [c, dst] is
computed as sum_b gt_b[:, :c].T @ band_b where gt blocks are dma_gathered
rows of a bf16 y-table (256B rows, AllGathered per layer) and band_b is the
0/1 segment matrix generated on DVE via tensor_scalar is_equal (4x mode).
Aggregation space per layer: L1 in 8ch (agg-first), L2 in 64ch (agg-first),
L3 in 128ch (agg-first). Gathers run on all 4 SWDGE queues in parallel
(~2.7ns/idx vs 8.6 on one queue). Per-node scales (dinv) are applied with a
host-replicated [128, NS] tile; the tiny MLP head is replicated.
"""
import sys
import types
from dataclasses import dataclass, field

import numpy as np
import ml_dtypes

import concourse.bass as bass
import concourse.tile as tile
from concourse import bacc, mybir
from concourse._compat import cdiv
from concourse.bass_utils import run_bass_kernel_spmd

P = 128
F32 = mybir.dt.float32
BF16 = mybir.dt.bfloat16
I16 = mybir.dt.int16
BF = ml_dtypes.bfloat16

N_NODES = 100000
N_CORES = 8
N_GRAPHS = 256
CHUNK = 25000
GW = 256          # dst group width (2 windows)
IN_CH = 8
C1, C2, C3 = 64, 128, 64
CH1 = 32
GSLOTS = 64


@dataclass
class Cfg:
    n_nodes: int = N_NODES
    n_cores: int = N_CORES

    @property
    def ns(self):
        return self.n_nodes // self.n_cores          # 12500

    @property
    def nw(self):
        return cdiv(self.ns, P)                       # 98

    @property
    def ng(self):
        return cdiv(self.ns, GW)                      # 49

    @property
    def n_chunks(self):
        return cdiv(self.n_nodes, CHUNK)              # 4

    @property
    def npad(self):
        return self.nw * P                            # 12544


@dataclass
class Plan:
    cfg: Cfg
    nb: np.ndarray                 # [NG, 4] blocks per cell (max over cores)
    g0: list = field(default_factory=list)
    gwid: list = field(default_factory=list)
    icol0: np.ndarray = None       # idx col offset per cell
    b0: np.ndarray = None          # seg block offset per cell
    idx_cols: int = 0
    nb_total: int = 0

    def finalize(self):
        ng, nk = self.nb.shape
        self.icol0 = np.zeros((ng, nk), dtype=np.int64)
        self.b0 = np.zeros((ng, nk), dtype=np.int64)
        icol = 0
        b = 0
        for g in range(ng):
            for k in range(nk):
                self.icol0[g, k] = icol
                self.b0[g, k] = b
                icol += int(self.nb[g, k]) * P // 16
                b += int(self.nb[g, k])
        self.idx_cols = icol
        self.nb_total = b


def preprocess(cfg: Cfg, edge_index, batch):
    src = np.asarray(edge_index[0], dtype=np.int64)
    dst = np.asarray(edge_index[1], dtype=np.int64)
    batch = np.asarray(batch, dtype=np.int64)
    NS, NG, NK = cfg.ns, cfg.ng, cfg.n_chunks

    deg = (np.bincount(dst, minlength=cfg.n_nodes) + 1.0).astype(np.float32)
    dinv = (1.0 / np.sqrt(deg)).astype(np.float32)
    cnt = np.bincount(batch, minlength=N_GRAPHS).astype(np.float32)
    invcnt = (1.0 / np.maximum(cnt, 1.0)).astype(np.float32)

    per_core = []
    counts = np.zeros((cfg.n_cores, NG, NK), dtype=np.int64)
    for c in range(cfg.n_cores):
        m = (dst >= c * NS) & (dst < (c + 1) * NS)
        s = src[m]
        d = dst[m] - c * NS
        g = d // GW
        k = s // CHUNK
        key = g * NK + k
        order = np.argsort(key, kind="stable")
        s, d, key = s[order], d[order], key[order]
        cnts = np.bincount(key, minlength=NG * NK).reshape(NG, NK)
        counts[c] = cnts
        starts = np.zeros(NG * NK + 1, dtype=np.int64)
        np.cumsum(cnts.reshape(-1), out=starts[1:])
        per_core.append((s, d, starts))

    nb = np.ceil(counts.max(axis=0) / P).astype(np.int64)   # [NG, NK]
    plan = Plan(Cfg(), nb)
    plan.g0 = [int(batch[c * NS]) for c in range(cfg.n_cores)]
    plan.gwid = [int(batch[(c + 1) * NS - 1]) - plan.g0[c] + 1
                 for c in range(cfg.n_cores)]
    assert max(plan.gwid) <= GSLOTS
    plan.finalize()

    shared = {
        "iota256": np.tile(np.arange(GW, dtype=np.float32),
                           (P, 1)).astype(BF),
        "iota64": np.tile(np.arange(GSLOTS, dtype=np.float32),
                          (P, 1)),
        "ident": np.eye(P, dtype=np.float32),
        "invcnt": np.tile(invcnt, (C3, 1)).astype(np.float32).reshape(
            C3, N_GRAPHS),
    }

    cores = []
    for c in range(cfg.n_cores):
        s, d, starts = per_core[c]
        idx_stream = np.zeros(plan.idx_cols * 16, dtype=np.int16)
        seg_stream = np.full(plan.nb_total * P, -1.0, dtype=np.float32)
        for g in range(NG):
            for k in range(NK):
                e0, e1 = starts[g * NK + k], starts[g * NK + k + 1]
                n = e1 - e0
                ipos = int(plan.icol0[g, k]) * 16
                bpos = int(plan.b0[g, k]) * P
                idx_stream[ipos:ipos + n] = (s[e0:e1] % CHUNK).astype(np.int16)
                seg_stream[bpos:bpos + n] = (d[e0:e1] % GW).astype(np.float32)
        idx_all = np.tile(idx_stream.reshape(-1, 16).T, (8, 1))
        seg_all = seg_stream.reshape(-1, P).T.astype(BF)

        nodes = np.arange(c * NS, (c + 1) * NS)
        dv = np.ones(cfg.npad, dtype=np.float32)
        dv[:NS] = dinv[nodes]
        dinv_rep = np.tile(dv, (P, 1)).astype(BF)
        gs = np.full(cfg.npad, -1.0, dtype=np.float32)
        gs[:NS] = (batch[nodes] - plan.g0[c]).astype(np.float32)
        gslot_t = gs.reshape(cfg.nw, P).T.copy()
        cores.append({
            "idx_all": idx_all, "seg_all": seg_all,
            "dinv_rep": dinv_rep, "gslot_t": gslot_t,
        })
    return plan, shared, cores


def build_program(plan: Plan, n_cores: int):
    cfg = plan.cfg
    NG, NK, NS, NW = cfg.ng, cfg.n_chunks, cfg.ns, cfg.nw
    nc = bacc.Bacc("TRN2", target_bir_lowering=False, debug=False,
                   num_devices=n_cores, num_swdge_queues=4)

    def din(name, shape, dt=F32):
        return nc.dram_tensor(name, shape, dt, kind="ExternalInput").ap()

    xT = din("xT", [IN_CH, cfg.npad])
    w1 = din("w1", [IN_CH, C1])
    w2 = din("w2", [C1, C2])
    w3 = din("w3", [C2, C3])
    wl1 = din("wl1", [C3, CH1])
    wl2 = din("wl2", [CH1, 1])
    b1c = din("b1c", [C1, 1])
    b2c = din("b2c", [C2, 1])
    b3c = din("b3c", [C3, 1])
    bl1c = din("bl1c", [CH1, 1])
    bl2c = din("bl2c", [1, 1])
    dinv_d = din("dinv_rep", [P, cfg.npad], BF16)
    gslot_d = din("gslot_t", [P, NW])
    iota256_d = din("iota256", [P, GW], BF16)
    iota64_d = din("iota64", [P, GSLOTS])
    ident_d = din("ident", [P, P])
    invcnt_d = din("invcnt", [C3, N_GRAPHS])
    idx_d = din("idx_all", [P, plan.idx_cols], I16)
    seg_d = din("seg_all", [P, plan.nb_total], BF16)
    out_d = nc.dram_tensor("out", [1, N_GRAPHS], F32,
                           kind="ExternalOutput").ap()
    DBG = bool(int(__import__("os").environ.get("GCN_DBG", "0")))
    if DBG:
        dbg_tab1 = nc.dram_tensor("dbg_tab1", [NS, P], BF16,
                                  kind="ExternalOutput").ap()
        nb00 = int(plan.nb[0, 0])
        dbg_gt = nc.dram_tensor("dbg_gt", [P, nb00, P], BF16,
                                kind="ExternalOutput").ap()
        dbg_band = nc.dram_tensor("dbg_band", [P, nb00, GW], BF16,
                                  kind="ExternalOutput").ap()
        dbg_ps = nc.dram_tensor("dbg_ps", [IN_CH, GW], F32,
                                kind="ExternalOutput").ap()
        dbg_yres = nc.dram_tensor("dbg_yres", [P, 12544], BF16,
                                  kind="ExternalOutput").ap()

    with tile.TileContext(nc) as tc:
        with tc.tile_pool(name="const", bufs=1) as cp, \
             tc.tile_pool(name="gath", bufs=5) as gp, \
             tc.tile_pool(name="band", bufs=3) as bp, \
             tc.tile_pool(name="win", bufs=2) as wp, \
             tc.tile_pool(name="psS", bufs=2, space="PSUM") as pps, \
             tc.tile_pool(name="psW", bufs=2, space="PSUM") as ppw, \
             tc.tile_pool(name="psT", bufs=1, space="PSUM") as ppt, \
             tc.tile_pool(name="psP", bufs=1, space="PSUM") as ppp, \
             tc.tile_pool(name="dram", bufs=1, space="DRAM") as dp:

            def load_const(name, ap, shape, dt=F32):
                t = cp.tile(shape, dt, tag=name)
                nc.sync.dma_start(t[:], ap[:])
                return t

            # f32 weights -> bf16 SBUF tiles
            def wbf(name, ap, shape):
                t32 = wp.tile(shape, F32, tag="wtmp")
                nc.sync.dma_start(t32[:], ap[:])
                t = cp.tile(shape, BF16, tag=name)
                nc.vector.tensor_copy(t[:], t32[:])
                return t

            w1_s = load_const("w1", w1, [IN_CH, C1])
            w2_s = load_const("w2", w2, [C1, C2])
            w3_s = load_const("w3", w3, [C2, C3])
            wl1_s = load_const("wl1", wl1, [C3, CH1])
            wl2_s = load_const("wl2", wl2, [CH1, 1])
            b1_s = load_const("b1", b1c, [C1, 1])
            b2_s = load_const("b2", b2c, [C2, 1])
            b3_s = load_const("b3", b3c, [C3, 1])
            bl1_s = load_const("bl1", bl1c, [CH1, 1])
            bl2_s = load_const("bl2", bl2c, [1, 1])
            dinv_s = load_const("dinv", dinv_d, [P, cfg.npad], BF16)
            gslot_s = load_const("gslot", gslot_d, [P, NW])
            iota256_s = load_const("iota256", iota256_d, [P, GW], BF16)
            iota64_s = load_const("iota64", iota64_d, [P, GSLOTS])
            ident_s = load_const("ident", ident_d, [P, P])
            invcnt_s = load_const("invcnt", invcnt_d, [C3, N_GRAPHS])
            xT_s = load_const("xT", xT, [IN_CH, cfg.npad])

            yres = cp.tile([P, cfg.npad], F32)
            nc.vector.memset(yres[:], 0.0)

            tab_sh = dp.tile([NS, P], BF16)
            tab_full = dp.tile([cfg.n_nodes, P], BF16)
            pool_sh = dp.tile([C3, GSLOTS], F32)
            pool_ag = dp.tile([C3 * n_cores, GSLOTS], F32)
            rg = [list(range(n_cores))]

            def ts(out, in0, s1, op0, s2=None, op1=mybir.AluOpType.bypass):
                nc.vector.tensor_scalar(out, in0, s1, s2, op0, op1)

            def store_table(cin, g):
                """Transpose yres[:cin, group cols] into tab_sh rows."""
                for wl in range(2):
                    w = g * 2 + wl
                    if w >= NW:
                        break
                    rows = NS - w * P if w == NW - 1 else P
                    tp = ppt.tile([P, P], F32, tag="tp")
                    nc.tensor.transpose(
                        tp[:, :cin], yres[:cin, w * P:(w + 1) * P],
                        ident_s[:cin, :cin])
                    tsb = wp.tile([P, P], BF16, tag="tsb")
                    nc.scalar.copy(tsb[:, :cin], tp[:, :cin])
                    nc.sync.dma_start(tab_sh[w * P:w * P + rows, :cin],
                                      tsb[:rows, :cin])

            def allgather():
                nc.gpsimd.collective_compute(
                    "AllGather", mybir.AluOpType.bypass, replica_groups=rg,
                    ins=[tab_sh.opt()], outs=[tab_full.opt()])

            # ---- prep: y1 = dinv * x (8ch), table1, AG ----
            nc.vector.tensor_tensor(yres[:IN_CH, :], xT_s[:],
                                    dinv_s[:IN_CH, :], mybir.AluOpType.mult)
            for g in range(NG):
                store_table(IN_CH, g)
            if DBG:
                nc.sync.dma_start(dbg_tab1[:], tab_sh[:])
            allgather()

            pool_ps = ppp.tile([C3, GSLOTS], F32)

            def agg_layer(lnum, cin, cout, w_s, b_s):
                last_layer = lnum == 3
                for g in range(NG):
                    ps = pps.tile([cin, GW], F32, tag="psS")
                    nblist = [int(plan.nb[g, k]) for k in range(NK)]
                    tot = sum(nblist)
                    bi = 0
                    for k in range(NK):
                        nbk = nblist[k]
                        if nbk == 0:
                            continue
                        ni = nbk * P
                        cols = ni // 16
                        c0 = int(plan.icol0[g, k])
                        it = wp.tile([P, cols], I16, tag=f"idx{k}")
                        nc.sync.dma_start(it[:], idx_d[:, c0:c0 + cols])
                        gt = gp.tile([P, nbk, P], BF16, tag="gt")
                        rlo = k * CHUNK
                        rhi = min(rlo + CHUNK, cfg.n_nodes)
                        nc.gpsimd.dma_gather(
                            gt[:], tab_full[rlo:rhi, :], it[:],
                            ni, ni, P, single_packet=False, queue_num=k)
                        sb0 = int(plan.b0[g, k])
                        seg_t = wp.tile([P, nbk], BF16, tag=f"seg{k}")
                        nc.sync.dma_start(seg_t[:], seg_d[:, sb0:sb0 + nbk])
                        band = bp.tile([P, nbk, GW], BF16, tag="band")
                        nc.vector.tensor_tensor(
                            out=band[:],
                            in0=seg_t[:].unsqueeze(2).broadcast_to([P, nbk, GW]),
                            in1=iota256_s[:].unsqueeze(1).broadcast_to(
                                [P, nbk, GW]),
                            op=mybir.AluOpType.is_equal)
                        if DBG and lnum == 1 and g == 0 and k == 0:
                            nc.sync.dma_start(dbg_gt[:], gt[:])
                            nc.sync.dma_start(dbg_band[:], band[:])
                        for b in range(nbk):
                            nc.tensor.matmul(
                                ps[:], lhsT=gt[:, b, :cin], rhs=band[:, b, :],
                                start=(bi == 0), stop=(bi == tot - 1),
                                skip_group_check=True)
                            bi += 1
                    # post: u = dinv * (S^T + y^T); h = relu(W^T u + b)
                    sl = slice(g * GW, (g + 1) * GW)
                    if DBG and lnum == 1 and g == 0:
                        pscp = wp.tile([IN_CH, GW], F32, tag="pscp")
                        nc.vector.tensor_copy(pscp[:], ps[:])
                        nc.sync.dma_start(dbg_ps[:], pscp[:])
                    t2 = wp.tile([cin, GW], F32, tag="t2")
                    nc.vector.tensor_tensor(t2[:], ps[:], yres[:cin, sl],
                                            mybir.AluOpType.add)
                    u = wp.tile([cin, GW], F32, tag="u")
                    nc.vector.tensor_tensor(u[:], t2[:], dinv_s[:cin, sl],
                                            mybir.AluOpType.mult)
                    ps2 = ppw.tile([cout, GW], F32, tag="psW")
                    nc.tensor.matmul(ps2[:], lhsT=w_s[:], rhs=u[:],
                                     start=True, stop=True)
                    h = wp.tile([cout, GW], F32, tag="h")
                    nc.scalar.activation(
                        h[:], ps2[:], mybir.ActivationFunctionType.Relu,
                        bias=b_s[:, 0:1])
                    if not last_layer:
                        nc.vector.tensor_tensor(
                            yres[:cout, sl], h[:], dinv_s[:cout, sl],
                            mybir.AluOpType.mult)
                        store_table(cout, g)
                    else:
                        for wl in range(2):
                            w = g * 2 + wl
                            if w >= NW:
                                break
                            tp = ppt.tile([P, P], F32, tag="tp")
                            nc.tensor.transpose(
                                tp[:, :cout], h[:, wl * P:(wl + 1) * P],
                                ident_s[:cout, :cout])
                            h3w = wp.tile([P, C3], F32, tag="h3w")
                            nc.scalar.copy(h3w[:], tp[:, :cout])
                            gb = wp.tile([P, GSLOTS], F32, tag="gb")
                            nc.vector.tensor_tensor(
                                out=gb[:],
                                in0=gslot_s[:, w:w + 1].broadcast_to(
                                    [P, GSLOTS]),
                                in1=iota64_s[:],
                                op=mybir.AluOpType.is_equal)
                            nc.tensor.matmul(
                                pool_ps[:], lhsT=h3w[:], rhs=gb[:],
                                start=(w == 0), stop=(w == NW - 1),
                                skip_group_check=True)

            agg_layer(1, IN_CH, C1, w1_s, b1_s)
            if DBG:
                nc.sync.dma_start(dbg_yres[:], yres[:])
            allgather()
            agg_layer(2, C1, C2, w2_s, b2_s)
            allgather()
            agg_layer(3, C2, C3, w3_s, b3_s)

            # ---- pooling finale + MLP (replicated) ----
            pool_sb = cp.tile([C3, GSLOTS], F32)
            nc.vector.tensor_copy(pool_sb[:], pool_ps[:])
            nc.sync.dma_start(pool_sh[:], pool_sb[:])
            nc.gpsimd.collective_compute(
                "AllGather", mybir.AluOpType.bypass, replica_groups=rg,
                ins=[pool_sh.opt()], outs=[pool_ag.opt()])
            M = cp.tile([C3, N_GRAPHS], F32)
            nc.vector.memset(M[:], 0.0)
            for c in range(n_cores):
                agc = wp.tile([C3, GSLOTS], F32, tag="agc")
                nc.sync.dma_start(agc[:], pool_ag[c * C3:(c + 1) * C3, :])
                g0 = plan.g0[c]
                wdt = min(plan.gwid[c], N_GRAPHS - g0)
                nc.vector.tensor_tensor(M[:, g0:g0 + wdt], M[:, g0:g0 + wdt],
                                        agc[:, :wdt], mybir.AluOpType.add)
            Mb = cp.tile([C3, N_GRAPHS], F32)
            nc.vector.tensor_tensor(Mb[:], M[:], invcnt_s[:],
                                    mybir.AluOpType.mult)
            ps1 = ppw.tile([CH1, N_GRAPHS], F32, tag="psW")
            nc.tensor.matmul(ps1[:], lhsT=wl1_s[:], rhs=Mb[:],
                             start=True, stop=True)
            g1 = cp.tile([CH1, N_GRAPHS], F32)
            nc.scalar.activation(g1[:], ps1[:],
                                 mybir.ActivationFunctionType.Relu,
                                 bias=bl1_s[:, 0:1])
            ps2 = ppw.tile([1, N_GRAPHS], F32, tag="psW")
            nc.tensor.matmul(ps2[:], lhsT=wl2_s[:], rhs=g1[:],
                             start=True, stop=True)
            osb = cp.tile([1, N_GRAPHS], F32)
            ts(osb[:], ps2[:], bl2_s[:, 0:1], mybir.AluOpType.add)
            nc.sync.dma_start(out_d[:], osb[:])

    nc.compile()
    return nc


def make_in_maps(cfg, plan, shared, cores, x, W1, b1, W2, b2, W3, b3,
                 Wl1, bl1, Wl2, bl2):
    NS = cfg.ns
    x = np.asarray(x, dtype=np.float32)
    com = {
        "w1": np.asarray(W1, np.float32), "w2": np.asarray(W2, np.float32),
        "w3": np.asarray(W3, np.float32),
        "wl1": np.asarray(Wl1, np.float32), "wl2": np.asarray(Wl2, np.float32),
        "b1c": np.asarray(b1, np.float32).reshape(-1, 1),
        "b2c": np.asarray(b2, np.float32).reshape(-1, 1),
        "b3c": np.asarray(b3, np.float32).reshape(-1, 1),
        "bl1c": np.asarray(bl1, np.float32).reshape(-1, 1),
        "bl2c": np.asarray(bl2, np.float32).reshape(1, 1),
        "iota256": shared["iota256"], "iota64": shared["iota64"],
        "ident": shared["ident"], "invcnt": shared["invcnt"],
    }
    in_maps = []
    for c in range(cfg.n_cores):
        m = dict(com)
        xs = np.zeros((IN_CH, cfg.npad), dtype=np.float32)
        xs[:, :NS] = x[c * NS:(c + 1) * NS].T
        m["xT"] = xs
        m.update(cores[c])
        in_maps.append(m)
    return in_maps


_CACHE = {}


def _install_profile_hook():
    try:
        import antenv.axon_hooks  # noqa: F401
        return
    except ImportError:
        pass
    try:
        mod = types.ModuleType("antenv.axon_hooks")
        _h = [None]
        mod.set_axon_ntff_profile_hook = lambda h: _h.__setitem__(0, h)
        mod.get_axon_ntff_profile_hook = lambda: _h[0]
        sys.modules["antenv.axon_hooks"] = mod
        from trn_agent_boot.trn_boot import _ntff_profile_via_ctypes
        mod.set_axon_ntff_profile_hook(
            _ntff_profile_via_ctypes("/opt/axon/libaxon_pjrt.so"))
    except Exception:
        pass


def run(cfg, x, edge_index, batch, W1, b1, W2, b2, W3, b3, Wl1, bl1, Wl2, bl2,
        trace=False):
    plan, shared, cores = preprocess(cfg, edge_index, batch)
    key = ("prog", plan.nb_total, plan.idx_cols,
           tuple(plan.g0), tuple(plan.gwid))
    if key not in _CACHE:
        _CACHE[key] = build_program(plan, cfg.n_cores)
    nc = _CACHE[key]
    in_maps = make_in_maps(cfg, plan, shared, cores, x, W1, b1, W2, b2,
                           W3, b3, Wl1, bl1, Wl2, bl2)
    if trace:
        _install_profile_hook()
    res = run_bass_kernel_spmd(nc, in_maps, list(range(cfg.n_cores)),
                               trace=trace)
    out = np.asarray(res.results[0]["out"]).reshape(-1)[:N_GRAPHS]
    return out.astype(np.float32), res


def kernel(x, edge_index, batch, W1, b1, W2, b2, W3, b3, Wl1, bl1, Wl2, bl2):
    cfg = Cfg()
    out, _ = run(cfg, x, edge_index, batch, W1, b1, W2, b2, W3, b3,
                 Wl1, bl1, Wl2, bl2)
    return out
